# revision 50
# baseline (speedup 1.0000x reference)
"""Trainium2 Bass kernel for CausalSelfAttention (B=4, T=2048, C=1024, H=16)
with additive prev-prob key bias.

Sharding: 8 cores = data-parallel over B (4) x tensor-parallel over head
halves (2).  Each core computes qkv for its 8 heads, causal attention, and a
partial output projection (row-parallel W_proj); host sums the two partials
per batch at unshard time.

Per-core device algorithm:
  - All matmul operands are bf16 (host-cast; PSUM accumulation stays fp32):
    1 cycle/row on the PE vs the multi-pass fp32 pipe, and well under the
    2e-2 harness gate (measured ~4.2e-3 vs ~3.3e-4 full-fp32).
  - K^T and Q^T kept feature-major with head pairs stacked in the 128
    partitions, so QK^T runs as two K=64 matmuls that the PE co-issues on
    row-group halves (h0/h64) — full-array throughput despite K=64.
  - Scores are computed transposed (keys on partitions): softmax denominator
    comes from an extra EA column appended to V (M=65 PV matmuls), where
    EA[k] = (prev_probs[k]+1e-10)**-EPS folds the additive log bias into a
    multiplicative per-key scale of exp(qk/8).
  - Causality: block-trimmed matmul widths + one 128x128 triangular mask
    multiply (bf16, DVE 2x rate) per diagonal block.
  - y^T layout feeds the output projection directly as the stationary
    operand. The per-(head,query) 1/denominator: mid-kernel pairs bounce the
    den row through DRAM (store / 32B-elem partition gather / vector
    reciprocal across 128 lanes / 32B-elem scatter / broadcast-read), all on
    the gpsimd SWDGE queue where the ~10us latency hides under later pairs'
    attention. The LAST pair computes it on-chip instead (K=1 matmuls spread
    the row over partitions, reciprocal, PE transpose-back, rank-1 ones
    broadcast) so the final projection isn't gated by DMA round-trips.
  - HAM awareness: the PE clock-gates to 1.2GHz after any ~3.4us idle
    window. Startup therefore front-loads chunk 0's x tiles (sync queue) and
    the qkv weights (gpsimd queue) in parallel and runs a ~3.4us dummy
    matmul burst so the first real matmuls start at 2.4GHz; the final
    projection allocates from the ps PSUM ring (not y) to avoid serializing
    behind the last pair's evacuation.
  - Next-chunk QKV generation and prev-chunk projection are emitted as
    resumable generator items, pulled into the attention loop (rationed
    ~items/4 per head pair) to fill PE stalls.
"""

import math
from contextlib import ExitStack

import ml_dtypes
import numpy as np

import concourse.bass as bass
import concourse.tile as tile
from concourse import bacc, mybir

F32 = mybir.dt.float32

# matmul operand dtype: bf16 streams the PE at 1 cycle/row (fp32/fp32r run
# the slow multi-pass fp32 pipe on hw and trip the 50% activity throttle);
# all accumulation stays fp32 in PSUM so only operand quantization is lost.
MMD = mybir.dt.bfloat16
NPMMD = ml_dtypes.bfloat16

B, T, C, H = 4, 2048, 1024, 16
HD = C // H          # 64
NCORES = 8
HPC = H // 2         # 8 heads per core
FPC = HPC * HD       # 512 features per core
NKT = T // 128       # 16 key tiles
NQC = T // 512       # 4 query chunks (also the x t-chunks)
NCT = C // 128       # 8 contraction tiles
EPS_BIAS = 0.1
SCALE = 1.0 / math.sqrt(HD)


def build(tc, out_ap, xT, wqkv, wproj, ea, tri_dram, id_dram, dsc1, dsc2):
    """Emit the per-core kernel into TileContext tc.

    out_ap : (T, C)    partial projection output (needs pair-sum on host)
    xT     : (C, T)    x[b] transposed
    wqkv   : (C, 3*FPC) [Wq_g | Wk_g | Wv_g] columns for this head group
    wproj  : (FPC, C)  W_proj rows for this head group
    ea     : (T,)      (prev_probs[b] + 1e-10) ** (-EPS_BIAS)
    tri_dram: (128,128) upper-triangular ones (tri[k,q] = 1 iff k <= q)
    dsc1/dsc2: (16, 1024) DRAM scratch for the denominator shuffle
    """
    nc = tc.nc
    ctx = tc.ctx
    Exp = mybir.ActivationFunctionType.Exp

    const = ctx.enter_context(tc.tile_pool(name="const", bufs=1))
    xs_pool = ctx.enter_context(tc.tile_pool(name="xs", bufs=9))
    qt_pool = ctx.enter_context(tc.tile_pool(name="qt", bufs=5))
    se_pool = ctx.enter_context(tc.tile_pool(name="se", bufs=4))
    tmp_pool = ctx.enter_context(tc.tile_pool(name="tmp", bufs=6))
    rec_pool = ctx.enter_context(tc.tile_pool(name="rec", bufs=4))
    scale_pool = ctx.enter_context(tc.tile_pool(name="scale", bufs=4))
    stack_pool = ctx.enter_context(tc.tile_pool(name="stack", bufs=8))
    pout_pool = ctx.enter_context(tc.tile_pool(name="pout", bufs=4))

    ps_pool = ctx.enter_context(tc.tile_pool(name="ps", bufs=2, space="PSUM"))
    st_pool = ctx.enter_context(tc.tile_pool(name="st", bufs=2, space="PSUM"))
    y_pool = ctx.enter_context(tc.tile_pool(name="y", bufs=2, space="PSUM"))


    # ---- constants / persistent buffers ----
    # Startup DMAs split across the two queues: chunk 0's x tiles go on the
    # sync queue (issued first, below), the qkv weights on the gpsimd SWDGE
    # queue, so the first Q matmul (needs x c-tile + wq c-tile) can start
    # ~9us in instead of waiting out 16 serialized transfers.
    tri = const.tile([128, 128], MMD, name="tri")
    eacol = const.tile([128, NKT], F32, name="eacol")
    wp_sb = const.tile([128, FPC // 128, C], MMD, name="wp_sb")     # 8KB/p

    # HAM warm-up fodder: a dep-free SBUF tile for dummy matmuls that keep
    # the PE activity monitor at K=8/8 (2.4GHz) through otherwise-idle spans
    warm = const.tile([128, 512], MMD, name="warm")
    nc.vector.memset(warm, 0.5)
    # small fp32 constants for the last pair's on-chip reciprocal path
    # (full-height so slices can match any operand's base partition)
    one1 = const.tile([128, 1], F32, name="one1")
    nc.vector.memset(one1, 1.0)
    ones64 = const.tile([128, 64], F32, name="ones64")
    nc.vector.memset(ones64, 1.0)
    idf32 = const.tile([128, 128], F32, name="idf32")

    wq_sb = const.tile([128, NCT, 3 * FPC], MMD, name="wq_sb")      # 24KB/p
    wqkv3 = wqkv.rearrange("(c p) f -> p c f", p=128)
    for c in range(NCT):
        nc.gpsimd.dma_start(out=wq_sb[:, c, :], in_=wqkv3[:, c, :])
    nc.gpsimd.dma_start(out=tri, in_=tri_dram[:, :])
    nc.gpsimd.dma_start(out=eacol, in_=ea.rearrange("(k p) -> p k", p=128))
    nc.gpsimd.dma_start(out=wp_sb, in_=wproj.rearrange("(i p) c -> p i c", p=128))
    nc.gpsimd.dma_start(out=idf32, in_=id_dram[:, :])

    kt = const.tile([128, HPC // 2, T], MMD, name="kt")             # 16KB/p
    v2 = const.tile([128, NKT, HPC, HD + 1], MMD, name="v2")        # 16.6KB/p
    ones8 = const.tile([128, HPC], F32, name="ones8")
    nc.vector.memset(ones8, 1.0)

    # EA columns of v2 (column HD of each head's slot): per-partition scalar
    # broadcast (EA value replicated across the 8 head slots)
    for kt_i in range(NKT):
        nc.vector.tensor_scalar(
            out=v2[:, kt_i, :, HD:HD + 1],
            in0=ones8.unsqueeze(2),
            scalar1=eacol[:, kt_i:kt_i + 1],
            scalar2=None,
            op0=mybir.AluOpType.mult,
        )

    prev_stacks = None

    def emit_proj(qc, stacks, pool=None, tag="ps", alt=False):
        # alt=True (final projection only): attention is over, so the st
        # PSUM ring is free — alternate tiles between ps and st rings
        # (effective depth 4 instead of 2) and alternate the output DMAs
        # across the sync/gpsimd queues so the 8 stores don't serialize.
        pool = pool or ps_pool
        for tq in range(4):
            row0 = qc * 512 + tq * 128
            for ch in range(2):
                i = tq * 2 + ch
                if alt and i % 2 == 1:
                    ps = st_pool.tile([128, 512], F32, tag="st",
                                      name=f"pps_{qc}_{tq}_{ch}")
                else:
                    ps = pool.tile([128, 512], F32, tag=tag,
                                   name=f"pps_{qc}_{tq}_{ch}")
                for p in range(HPC // 2):
                    nc.tensor.matmul(
                        ps,
                        stacks[p][:, tq * 128:(tq + 1) * 128],
                        wp_sb[:, p, ch * 512:(ch + 1) * 512],
                        start=(p == 0),
                        stop=(p == HPC // 2 - 1),
                    )
                pout = pout_pool.tile([128, 512], F32, tag="pout",
                                      name=f"po_{qc}_{tq}_{ch}")
                nc.vector.tensor_copy(pout, ps)
                dma_q = nc.gpsimd if (alt and i % 2 == 1) else nc.sync
                dma_q.dma_start(
                    out=out_ap[row0:row0 + 128, ch * 512:(ch + 1) * 512],
                    in_=pout,
                )

    qts_store = {}
    xs_store = {}

    def gen_chunk(qc):
        """Emit one t-chunk's pre-attention work as resumable items:
        x loads, JIT Q^T, K^T tiles, V tiles."""
        xs_tiles = []
        for c in range(NCT):
            xs = xs_pool.tile([128, 512], MMD, tag="xs", name=f"xs_{qc}_{c}")
            nc.sync.dma_start(
                out=xs, in_=xT[c * 128:(c + 1) * 128, qc * 512:(qc + 1) * 512]
            )
            xs_tiles.append(xs)
        xs_store[qc] = xs_tiles
        yield
        qts = []
        for p in range(HPC // 2):
            ps = ps_pool.tile([128, 512], F32, tag="ps", name=f"qps_{qc}_{p}")
            for c in range(NCT):
                nc.tensor.matmul(
                    ps,
                    wq_sb[:, c, p * 128:(p + 1) * 128],
                    xs_tiles[c],
                    start=(c == 0),
                    stop=(c == NCT - 1),
                )
            qt = qt_pool.tile([128, 512], MMD, tag="qt", name=f"qt_{qc}_{p}")
            nc.vector.tensor_copy(qt, ps)
            qts.append(qt)
            yield
        qts_store[qc] = qts
        for p in range(HPC // 2):
            ps = ps_pool.tile([128, 512], F32, tag="ps", name=f"kps_{qc}_{p}")
            for c in range(NCT):
                nc.tensor.matmul(
                    ps,
                    wq_sb[:, c, FPC + p * 128:FPC + (p + 1) * 128],
                    xs_tiles[c],
                    start=(c == 0),
                    stop=(c == NCT - 1),
                )
            nc.vector.tensor_copy(kt[:, p, qc * 512:(qc + 1) * 512], ps)
            yield
        for j in range(4):
            kt_i = qc * 4 + j
            ps = ps_pool.tile([128, 512], F32, tag="ps", name=f"vps_{qc}_{j}")
            for c in range(NCT):
                nc.tensor.matmul(
                    ps,
                    xs_tiles[c][:, j * 128:(j + 1) * 128],
                    wq_sb[:, c, 2 * FPC:3 * FPC],
                    start=(c == 0),
                    stop=(c == NCT - 1),
                )
            nc.vector.tensor_scalar(
                out=v2[:, kt_i, :, 0:HD],
                in0=ps.rearrange("p (h d) -> p h d", h=HPC),
                scalar1=eacol[:, kt_i:kt_i + 1],
                scalar2=None,
                op0=mybir.AluOpType.mult,
            )
            yield

    def gen_proj(qc, stacks):
        for tq in range(4):
            row0 = qc * 512 + tq * 128
            for ch in range(2):
                ps = ps_pool.tile([128, 512], F32, tag="ps",
                                  name=f"pps_{qc}_{tq}_{ch}")
                for p in range(HPC // 2):
                    nc.tensor.matmul(
                        ps,
                        stacks[p][:, tq * 128:(tq + 1) * 128],
                        wp_sb[:, p, ch * 512:(ch + 1) * 512],
                        start=(p == 0),
                        stop=(p == HPC // 2 - 1),
                    )
                pout = pout_pool.tile([128, 512], F32, tag="pout",
                                      name=f"po_{qc}_{tq}_{ch}")
                nc.vector.tensor_copy(pout, ps)
                nc.sync.dma_start(
                    out=out_ap[row0:row0 + 128, ch * 512:(ch + 1) * 512],
                    in_=pout,
                )
                yield

    # Stage startup for the shortest path to the first real matmul: chunk 0's
    # x tiles go onto the sync queue FIRST, then the qkv weights; meanwhile
    # ~3.4us of dummy matmuls trip the HAM clock gate to 2.4GHz so the real
    # stream starts warm.
    gen0 = gen_chunk(0)
    next(gen0)  # chunk 0 xs DMAs onto the sync queue
    wps = ps_pool.tile([128, 512], F32, tag="ps", name="warm_start")
    for _ in range(8):
        nc.tensor.matmul(wps, warm[:, 0:128], warm, start=True, stop=True,
                         skip_group_check=True)
    for _ in gen0:
        pass

    for qc in range(NQC):
        fillers = []
        if qc + 1 < NQC:
            fillers.append(gen_chunk(qc + 1))
        if prev_stacks is not None:
            fillers.append(gen_proj(qc - 1, prev_stacks))

        def pull(n):
            for _ in range(n):
                while fillers:
                    try:
                        next(fillers[0])
                        break
                    except StopIteration:
                        fillers.pop(0)

        # ---- attention for this query chunk, per head pair ----
        stacks = []
        nki = 4 * qc + 4
        # ration filler pulls to ~items/4 per head pair, front-loaded within
        # each pair: the pair-start QK->exp latency is where the PE starves
        pull_stride = max(1, (nki + 4) // 5)
        qts = qts_store[qc]
        for p in range(HPC // 2):
            qt = qts[p]
            yA = y_pool.tile([128, 512], F32, tag="y", name=f"yA_{qc}_{p}")
            yB = y_pool.tile([128, 512], F32, tag="y", name=f"yB_{qc}_{p}")
            for ki in range(nki):
                r = ki - 4 * qc  # >= 0 on the block diagonal
                n0 = 128 * r if r > 0 else 0
                st = st_pool.tile([128, 1024], F32, tag="st", name=f"st_{qc}_{p}_{ki}")
                st3 = st.rearrange("p (h q) -> p h q", h=2)
                kslice = slice(ki * 128, (ki + 1) * 128)
                nc.tensor.matmul(
                    st3[:, 0, n0:512], kt[0:64, p, kslice], qt[0:64, n0:512],
                    start=True, stop=True,
                )
                nc.tensor.matmul(
                    st3[:, 1, n0:512], kt[64:128, p, kslice], qt[64:128, n0:512],
                    start=True, stop=True,
                )
                se = se_pool.tile([128, 1024], MMD, tag="se", name=f"se_{qc}_{p}_{ki}")
                se3 = se.rearrange("p (h q) -> p h q", h=2)
                if ki <= 1:
                    # split the pair's first exp per head: PV-A only waits
                    # ~half the ACT latency, and exp-B overlaps PV-A —
                    # shrinks the pair-start PE bubble when fillers are dry
                    nc.scalar.activation(
                        se3[:, 0, n0:512], st3[:, 0, n0:512], Exp, scale=SCALE
                    )
                    nc.scalar.activation(
                        se3[:, 1, n0:512], st3[:, 1, n0:512], Exp, scale=SCALE
                    )
                else:
                    nc.scalar.activation(
                        se3[:, :, n0:512], st3[:, :, n0:512], Exp, scale=SCALE
                    )
                if r >= 0:
                    nc.vector.tensor_mul(
                        se3[:, 0, n0:n0 + 128], se3[:, 0, n0:n0 + 128], tri
                    )
                    nc.vector.tensor_mul(
                        se3[:, 1, n0:n0 + 128], se3[:, 1, n0:n0 + 128], tri
                    )
                nc.tensor.matmul(
                    yA[0:HD + 1, n0:512], v2[:, ki, 2 * p, :], se3[:, 0, n0:512],
                    start=(ki == 0), stop=(ki == nki - 1), skip_group_check=True,
                )
                nc.tensor.matmul(
                    yB[0:HD + 1, n0:512], v2[:, ki, 2 * p + 1, :], se3[:, 1, n0:512],
                    start=(ki == 0), stop=(ki == nki - 1), skip_group_check=True,
                )
                if ki % pull_stride == 0:
                    pull(1)

            # evacuate y^T + denominators, normalize, stack the head pair
            # (on vector: the scalar/ACT engine is saturated by the exps)
            tmpA = tmp_pool.tile([128, 512], F32, tag="tmp", name=f"tmpA_{qc}_{p}")
            nc.vector.tensor_copy(tmpA[0:HD + 1, :], yA[0:HD + 1, :])
            tmpB = tmp_pool.tile([128, 512], F32, tag="tmp", name=f"tmpB_{qc}_{p}")
            nc.vector.tensor_copy(tmpB[0:HD + 1, :], yB[0:HD + 1, :])

            if qc == NQC - 1 and p == HPC // 2 - 1:
                # Last pair: the ~11us DRAM bounce would idle the PE past the
                # HAM window and gate the final projection. Compute 1/den
                # on-chip instead: K=1 matmuls spread the den rows across
                # partitions, vector reciprocal, transpose back via the PE,
                # and a rank-1 ones matmul broadcasts 1/den to 64 partitions.
                dps = y_pool.tile([128, 8], F32, tag="y", name="dps")
                for j in range(4):
                    nc.tensor.matmul(
                        dps[:, j:j + 1],
                        tmpA[HD:HD + 1, j * 128:(j + 1) * 128],
                        one1[HD:HD + 1, :],
                        start=True, stop=True, skip_group_check=True,
                    )
                    nc.tensor.matmul(
                        dps[:, 4 + j:5 + j],
                        tmpB[HD:HD + 1, j * 128:(j + 1) * 128],
                        one1[HD:HD + 1, :],
                        start=True, stop=True, skip_group_check=True,
                    )
                rcp8 = rec_pool.tile([128, 8], F32, tag="rcp", name="rcp_last")
                nc.vector.reciprocal(rcp8, dps)
                rrowA = y_pool.tile([1, 512], F32, tag="y", name="rrowA")
                rrowB = y_pool.tile([1, 512], F32, tag="y", name="rrowB")
                for j in range(8):
                    h, jj = divmod(j, 4)
                    nc.tensor.matmul(
                        (rrowB if h else rrowA)[0:1, jj * 128:(jj + 1) * 128],
                        rcp8[:, j:j + 1], idf32,
                        start=True, stop=True, skip_group_check=True,
                    )
                rsbA = rec_pool.tile([1, 512], F32, tag="rsbA", name="rsbA")
                nc.vector.tensor_copy(rsbA, rrowA)
                rsbB = rec_pool.tile([1, 512], F32, tag="rsbB", name="rsbB")
                nc.vector.tensor_copy(rsbB, rrowB)
                scA = y_pool.tile([64, 512], F32, tag="y", name="scA")
                nc.tensor.matmul(scA, ones64[0:1, :], rsbA,
                                 start=True, stop=True, skip_group_check=True)
                scB = y_pool.tile([64, 512], F32, tag="y", name="scB")
                nc.tensor.matmul(scB, ones64[0:1, :], rsbB,
                                 start=True, stop=True, skip_group_check=True)
                srcA, srcB = scA[0:64, :], scB[0:64, :]
            else:
                # 1/denominator: bounce rows through DRAM to spread the 1024
                # values over 128 partitions (reciprocal runs at 8 cyc/elem
                # on the free dim), then broadcast straight from DRAM (step-0
                # partition source is legal for DRAM).
                idx = qc * 4 + p

                # the two den-row stores ride different queues so the gather
                # waits max(A, B) rather than their sum
                nc.gpsimd.dma_start(out=dsc1[idx, 0:512], in_=tmpA[HD:HD + 1, :])
                nc.sync.dma_start(out=dsc1[idx, 512:1024], in_=tmpB[HD:HD + 1, :])
                # lane order is irrelevant to the elementwise reciprocal: the
                # contiguous (p j) split gives 32B per partition per hop
                # instead of 4B descriptors (which measured ~9.6us each)
                dnp = rec_pool.tile([128, 8], F32, tag="dnp", name=f"dnp_{qc}_{p}")
                nc.gpsimd.dma_start(
                    out=dnp, in_=dsc1[idx, :].rearrange("(p j) -> p j", p=128)
                )
                rcp = rec_pool.tile([128, 8], F32, tag="rcp", name=f"rcp_{qc}_{p}")
                nc.vector.reciprocal(rcp, dnp)
                nc.gpsimd.dma_start(
                    out=dsc2[idx, :].rearrange("(p j) -> p j", p=128), in_=rcp
                )
                sc = scale_pool.tile([64, 1024], F32, tag="scale", name=f"sc_{qc}_{p}")
                nc.gpsimd.dma_start(
                    out=sc[0:64, :], in_=dsc2[idx:idx + 1, :].to_broadcast([64, 1024])
                )
                srcA, srcB = sc[0:64, 0:512], sc[0:64, 512:1024]

            stack = stack_pool.tile([128, 512], MMD, tag="stack", name=f"stk_{qc}_{p}")
            nc.vector.tensor_mul(stack[0:64, :], tmpA[0:64, :], srcA)
            stkB = tmp_pool.tile([64, 512], MMD, tag="stkB", bufs=3,
                                 name=f"skB_{qc}_{p}")
            nc.vector.tensor_mul(stkB[0:64, :], tmpB[0:64, :], srcB)
            nc.sync.dma_start(out=stack[64:128, :], in_=stkB[0:64, :])
            stacks.append(stack)
            pull(2)

        pull(1000)
        prev_stacks = stacks
    # Final projection on the now-free ps pool (y-pool reuse would serialize
    # it behind the last pair's PSUM evacuation).
    emit_proj(NQC - 1, prev_stacks, alt=True)


def make_nc():
    nc = bacc.Bacc("TRN2", target_bir_lowering=False, debug=False,
                   num_devices=NCORES)
    xT = nc.dram_tensor("xT", [C, T], MMD, kind="ExternalInput")
    wqkv = nc.dram_tensor("wqkv", [C, 3 * FPC], MMD, kind="ExternalInput")
    wproj = nc.dram_tensor("wproj", [FPC, C], MMD, kind="ExternalInput")
    ea = nc.dram_tensor("ea", [T], F32, kind="ExternalInput")
    out = nc.dram_tensor("out", [T, C], F32, kind="ExternalOutput")
    dsc1 = nc.dram_tensor("dsc1", [16, 1024], F32, kind="Internal")
    dsc2 = nc.dram_tensor("dsc2", [16, 1024], F32, kind="Internal")
    tri_np = np.triu(np.ones((128, 128), dtype=NPMMD))
    tri_dram = nc.inline_tensor(tri_np, name="tri_const")
    id_dram = nc.inline_tensor(np.eye(128, dtype=np.float32), name="id_const")
    with ExitStack() as ctx:
        tc = ctx.enter_context(tile.TileContext(nc))
        tc.ctx = ctx
        build(tc, out[:, :], xT[:, :], wqkv[:, :], wproj[:, :], ea[:],
              tri_dram, id_dram, dsc1[:, :], dsc2[:, :])
    nc.compile()
    return nc


def shard_inputs(x, prev_probs, W_attn, W_proj):
    in_maps = []
    for core in range(NCORES):
        b, g = divmod(core, 2)
        xT = np.ascontiguousarray(x[b].T)
        wq = W_attn[:, g * FPC:(g + 1) * FPC]
        wk = W_attn[:, C + g * FPC:C + (g + 1) * FPC]
        wv = W_attn[:, 2 * C + g * FPC:2 * C + (g + 1) * FPC]
        wqkv = np.ascontiguousarray(np.concatenate([wq, wk, wv], axis=1))
        wproj = np.ascontiguousarray(W_proj[g * FPC:(g + 1) * FPC, :])
        ea = np.power(prev_probs[b] + np.float32(1e-10), np.float32(-EPS_BIAS))
        in_maps.append(
            {
                "xT": xT.astype(NPMMD),
                "wqkv": wqkv.astype(NPMMD),
                "wproj": wproj.astype(NPMMD),
                "ea": ea.astype(np.float32),
            }
        )
    return in_maps


_CACHED_NC = None


def kernel(x, prev_probs, W_attn, W_proj, trace=False, tmpdir=None):
    global _CACHED_NC
    from concourse.bass_utils import run_bass_kernel_spmd

    x = np.asarray(x, dtype=np.float32)
    prev_probs = np.asarray(prev_probs, dtype=np.float32)
    W_attn = np.asarray(W_attn, dtype=np.float32)
    W_proj = np.asarray(W_proj, dtype=np.float32)

    if _CACHED_NC is None:
        _CACHED_NC = make_nc()
    nc = _CACHED_NC

    in_maps = shard_inputs(x, prev_probs, W_attn, W_proj)
    res = run_bass_kernel_spmd(
        nc, in_maps, core_ids=list(range(NCORES)), trace=trace, tmpdir=tmpdir
    )
    parts = [r["out"] for r in res.results]
    out = np.empty((B, T, C), dtype=np.float32)
    for b in range(B):
        out[b] = parts[2 * b] + parts[2 * b + 1]
    kernel.last_results = res
    return out

